# revision 9
# baseline (speedup 1.0000x reference)
"""Trainium2 Bass kernel for nn_LocationSemanticModel.

Data-parallel over batch: 8 cores x 32 batch each. Inside each core:
  - encoder: dma_gather(transpose) of loc/tim embeddings (f16, H-major),
    lerp with host-precomputed attention weights -> rep.T [128, 2, L*32]
  - GRU scan: GI = Wih@rep.T accumulated in PSUM per 8-step block (PE),
    recurrent Whh@h.T accumulates on top; gates on ACT/DVE/GPSIMD.
  - weighted hidden sum via big mul+reduce with host-precomputed stw.
  - fc: col-tiled matmul vs fc_W.T (f16) + host-gathered spa_dis rows.
No collectives; host concatenates per-core [32, NLOC] outputs.
"""

import os
import sys

import numpy as np

for _p in ("/opt/trn_rl_repo",):
    if _p not in sys.path and os.path.isdir(_p):
        sys.path.insert(0, _p)

import concourse.bass as bass  # noqa: E402
from concourse.bacc import Bacc  # noqa: E402
import concourse.mybir as mybir  # noqa: E402
from concourse.tile import TileContext  # noqa: E402

F16 = mybir.dt.float16
F32 = mybir.dt.float32
I16 = mybir.dt.int16

B, L, H, NLOC, NTIME = 256, 400, 256, 10000, 169
NC = 8
BC = B // NC          # 32 batch per core
HC = 2                # 256 = 2 x 128 hidden chunks
MC = 6                # 768 = 6 x 128 gate chunks
TB = 8                # scan block: GI steps per psum buffer
TE = 16               # encoder block steps: 16*32=512 idxs = gather limit
NFC = 500             # fc N-chunk (<=512 f32 psum bank)
AluOp = mybir.AluOpType
Act = mybir.ActivationFunctionType


def build_kernel(l_steps=L, bc=BC, debug=False):
    """Build the per-core SPMD Bass program. Returns nc."""
    nb = l_steps // TB            # scan blocks
    ne = l_steps // TE if l_steps % TE == 0 else 1   # encoder blocks
    te = TE if l_steps % TE == 0 else l_steps
    items = l_steps * bc
    nfc_chunks = NLOC // 4 // NFC  # 2500/500 = 5
    assert items % 128 == 0 and NLOC % 4 == 0 and (NLOC // 4) % NFC == 0

    nc = Bacc()

    # ---- DRAM parameters (per-core inputs) ----
    loc_emb = nc.declare_dram_parameter("loc_emb16", [NLOC, H], F16, isOutput=False)
    tim_emb = nc.declare_dram_parameter("tim_emb16", [NTIME, H], F16, isOutput=False)
    wihT = nc.declare_dram_parameter("wihT16", [H, 3 * H], F16, isOutput=False)
    whhT = nc.declare_dram_parameter("whhT16", [H, 3 * H], F16, isOutput=False)
    fcWT = nc.declare_dram_parameter("fcWT16", [H, NLOC], F16, isOutput=False)
    loc_idx = nc.declare_dram_parameter("loc_idx", [16, items // 16], I16, isOutput=False)
    tim_idx = nc.declare_dram_parameter("tim_idx", [16, items // 16], I16, isOutput=False)
    w_att = nc.declare_dram_parameter("w_att", [128, items], F16, isOutput=False)
    stw_b = nc.declare_dram_parameter("stw_b", [128, bc * l_steps], F16, isOutput=False)
    spa4 = nc.declare_dram_parameter("spa4", [128, NLOC // 4], F32, isOutput=False)
    out4 = nc.declare_dram_parameter("out4", [128, NLOC // 4], F32, isOutput=True)
    if debug:
        dbg_rep = nc.declare_dram_parameter("dbg_rep", [128, HC, l_steps * bc], F16, isOutput=True)
        dbg_hall = nc.declare_dram_parameter("dbg_hall", [128, HC, bc, l_steps], F16, isOutput=True)
        dbg_ss = nc.declare_dram_parameter("dbg_ss", [128, HC, bc], F32, isOutput=True)

    with TileContext(nc) as tc:
        with (
            tc.tile_pool(name="const", bufs=1) as cpool,
            tc.tile_pool(name="rep", bufs=1) as rep_pool,
            tc.tile_pool(name="hall", bufs=1) as hall_pool,
        ):
            # ---- persistent SBUF ----
            wih_sb = cpool.tile([128, HC, 3 * H], F16, tag="wih")
            whh_sb = cpool.tile([128, HC, 3 * H], F16, tag="whh")
            nc.sync.dma_start(
                out=wih_sb[:], in_=wihT.rearrange("(c p) m -> p c m", p=128)
            )
            nc.sync.dma_start(
                out=whh_sb[:], in_=whhT.rearrange("(c p) m -> p c m", p=128)
            )
            lidx_sb = cpool.tile([128, items // 16], I16, tag="lidx")
            tidx_sb = cpool.tile([128, items // 16], I16, tag="tidx")
            nc.vector.memset(lidx_sb[:], 0)
            nc.vector.memset(tidx_sb[:], 0)
            # sim reads idx wrap from partitions [0:16); HW (queue 0 tx
            # core) reads [16:32) — populate both.
            nc.sync.dma_start(out=lidx_sb[:16, :], in_=loc_idx[:, :])
            nc.sync.dma_start(out=tidx_sb[:16, :], in_=tim_idx[:, :])
            nc.sync.dma_start(out=lidx_sb[16:32, :], in_=loc_idx[:, :])
            nc.sync.dma_start(out=tidx_sb[16:32, :], in_=tim_idx[:, :])

            rep_sb = rep_pool.tile([128, HC, items], F16, tag="rep")
            h_all = hall_pool.tile([128, HC, bc, l_steps], F16, tag="hall")

            # ---- encoder: gather + lerp ----
            with (
                tc.tile_pool(name="enc", bufs=2) as epool,
                tc.tile_pool(name="encw", bufs=2) as wpool,
            ):
                for e in range(ne):
                    i0 = e * te * bc
                    n_i = te * bc
                    locT = epool.tile([128, HC, n_i], F16, tag="locT")
                    timT = epool.tile([128, HC, n_i], F16, tag="timT")
                    # dma_gather transpose crashes above 512 idxs/instr
                    assert n_i <= 512
                    nc.gpsimd.dma_gather(
                        out_ap=locT[:],
                        in_ap=loc_emb[:],
                        idxs_ap=lidx_sb[:, i0 // 16 : (i0 + n_i) // 16],
                        num_idxs=n_i,
                        num_idxs_reg=n_i,
                        elem_size=H,
                        transpose=True,
                    )
                    nc.gpsimd.dma_gather(
                        out_ap=timT[:],
                        in_ap=tim_emb[:],
                        idxs_ap=tidx_sb[:, i0 // 16 : (i0 + n_i) // 16],
                        num_idxs=n_i,
                        num_idxs_reg=n_i,
                        elem_size=H,
                        transpose=True,
                    )
                    w_sb = wpool.tile([128, n_i], F16, tag="w")
                    nc.sync.dma_start(out=w_sb[:], in_=w_att[:, i0 : i0 + n_i])
                    # rep = loc + w*(tim - loc)
                    dlt = epool.tile([128, HC, n_i], F16, tag="dlt")
                    nc.gpsimd.tensor_sub(out=dlt[:], in0=timT[:], in1=locT[:])
                    for hc in range(HC):
                        nc.vector.tensor_mul(
                            out=dlt[:, hc, :], in0=dlt[:, hc, :], in1=w_sb[:]
                        )
                    nc.vector.tensor_add(
                        out=rep_sb[:, :, i0 : i0 + n_i], in0=locT[:], in1=dlt[:]
                    )

            # ---- GRU scan ----
            with (
                tc.tile_pool(name="gi_ps", bufs=2, space="PSUM") as gi_pool,
                tc.tile_pool(name="hn_ps", bufs=2, space="PSUM") as hn_pool,
                tc.tile_pool(name="hbuf", bufs=2) as hpool,
                tc.tile_pool(name="gate", bufs=3) as gpool,
            ):
                h_cur = hpool.tile([128, HC, bc], F16, tag="h")
                nc.vector.memset(h_cur[:], 0)
                for kb in range(nb):
                    t0 = kb * TB
                    gi = gi_pool.tile([128, MC, TB * bc], F32, tag="gi", space="PSUM")
                    hn = hn_pool.tile([128, HC, TB * bc], F32, tag="hn", space="PSUM")
                    # GI block: gi[mc] += WihT[kc,mc].T @ rep[kc, block]
                    for mc in range(MC):
                        for kc in range(HC):
                            nc.tensor.matmul(
                                out=gi[:, mc, :],
                                lhsT=wih_sb[:, kc, mc * 128 : (mc + 1) * 128],
                                rhs=rep_sb[:, kc, t0 * bc : (t0 + TB) * bc],
                                start=(kc == 0),
                                stop=(kc == HC - 1),
                                skip_group_check=True,
                            )
                    for s in range(TB):
                        t = t0 + s
                        sl = slice(s * bc, (s + 1) * bc)
                        # recurrent matmuls: r,z accumulate into gi; hn separate
                        for mc in range(MC):
                            dst = gi[:, mc, sl] if mc < 4 else hn[:, mc - 4, sl]
                            for kc in range(HC):
                                nc.tensor.matmul(
                                    out=dst,
                                    lhsT=whh_sb[:, kc, mc * 128 : (mc + 1) * 128],
                                    rhs=h_cur[:, kc, :],
                                    start=(mc >= 4 and kc == 0),
                                    stop=(mc >= 4 and kc == HC - 1),
                                    skip_group_check=True,
                                )
                        rz = gpool.tile([128, 4, bc], F16, tag="rz")
                        nc.scalar.activation(rz[:], gi[:, 0:4, sl], Act.Sigmoid)
                        t1 = gpool.tile([128, HC, bc], F16, tag="t1")
                        nc.vector.tensor_mul(out=t1[:], in0=rz[:, 0:2, :], in1=hn[:, :, sl])
                        t2 = gpool.tile([128, HC, bc], F16, tag="t2")
                        nc.vector.tensor_add(out=t2[:], in0=t1[:], in1=gi[:, 4:6, sl])
                        n16 = gpool.tile([128, HC, bc], F16, tag="n16")
                        nc.scalar.activation(n16[:], t2[:], Act.Tanh)
                        d16 = gpool.tile([128, HC, bc], F16, tag="d16")
                        nc.gpsimd.tensor_sub(out=d16[:], in0=h_cur[:], in1=n16[:])
                        e16 = gpool.tile([128, HC, bc], F16, tag="e16")
                        nc.gpsimd.tensor_mul(out=e16[:], in0=d16[:], in1=rz[:, 2:4, :])
                        h_nxt = hpool.tile([128, HC, bc], F16, tag="h")
                        nc.vector.tensor_add(out=h_nxt[:], in0=e16[:], in1=n16[:])
                        nc.gpsimd.tensor_copy(out=h_all[:, :, :, t], in_=h_nxt[:])
                        h_cur = h_nxt

            if debug:
                nc.sync.dma_start(out=dbg_rep[:], in_=rep_sb[:])
                nc.sync.dma_start(out=dbg_hall[:], in_=h_all[:])
            # ---- weighted sum + fc ----
            with (
                tc.tile_pool(name="fin", bufs=1) as fpool,
                tc.tile_pool(name="fc_ps", bufs=2, space="PSUM") as fc_pool,
            ):
                stw_sb = fpool.tile([128, bc, l_steps], F16, tag="stw")
                nc.sync.dma_start(out=stw_sb[:], in_=stw_b[:, :])
                fcw_sb = fpool.tile([128, HC, NLOC], F16, tag="fcw")
                nc.sync.dma_start(
                    out=fcw_sb[:], in_=fcWT.rearrange("(c p) m -> p c m", p=128)
                )
                spa_sb = fpool.tile([128, NLOC // 4], F32, tag="spa")
                nc.sync.dma_start(out=spa_sb[:], in_=spa4[:, :])

                for hc in range(HC):
                    nc.vector.tensor_mul(
                        out=h_all[:, hc, :, :], in0=h_all[:, hc, :, :], in1=stw_sb[:]
                    )
                ss = fpool.tile([128, HC, bc], F32, tag="ss")
                nc.vector.tensor_reduce(
                    out=ss[:], in_=h_all[:], axis=mybir.AxisListType.X, op=AluOp.add
                )
                if debug:
                    nc.sync.dma_start(out=dbg_ss[:], in_=ss[:])
                ss16 = fpool.tile([128, HC, bc], F16, tag="ss16")
                nc.vector.tensor_copy(out=ss16[:], in_=ss[:])

                out_sb = fpool.tile([128, NLOC // 4], F32, tag="out")
                for nck in range(nfc_chunks):
                    ps = fc_pool.tile([128, 512], F32, tag="fps", space="PSUM")
                    for kc in range(HC):
                        for j in range(4):
                            nc.tensor.matmul(
                                out=ps[32 * j : 32 * j + bc, :NFC],
                                lhsT=ss16[:, kc, :],
                                rhs=fcw_sb[
                                    :,
                                    kc,
                                    j * (NLOC // 4)
                                    + nck * NFC : j * (NLOC // 4)
                                    + (nck + 1) * NFC,
                                ],
                                start=(kc == 0),
                                stop=(kc == HC - 1),
                                tile_position=(0, 32 * j),
                                skip_group_check=True,
                            )
                    nsl = slice(nck * NFC, (nck + 1) * NFC)
                    nc.vector.tensor_add(out=out_sb[:, nsl], in0=spa_sb[:, nsl], in1=ps[:, :NFC])
                nc.sync.dma_start(out=out4[:, :], in_=out_sb[:])

    nc.finalize()
    return nc


# ------------------------- host side -------------------------

_CACHE = {}


def _sigmoid(x):
    return 1.0 / (1.0 + np.exp(-x))


def _prep(x, locOneHot, valLen, spa_dis_mat_e, loc_emb, tim_emb,
          attn_W1, attn_b1, attn_W2, attn_b2,
          gru_Wih, gru_Whh, gru_bih, gru_bhh, fc_W, fc_b):
    f = np.float32
    assert np.abs(gru_bih).max() == 0.0 and np.abs(gru_bhh).max() == 0.0, (
        "kernel assumes zero GRU biases"
    )
    l_steps = x.shape[1]
    slot = x[:, :, 4].astype(np.int32)
    lidx = locOneHot[:, :, 0].astype(np.int32)
    t_sec = x[:, :, 2].astype(f)
    spa = x[:, :, 0:2].astype(f)
    vl = valLen.astype(np.int32)

    # attention score tables (softmax over pair == sigmoid of diff)
    def s_table(emb):
        p = np.maximum(emb.astype(f) @ attn_W1.astype(f) + attn_b1.astype(f), 0.0)
        return (p @ attn_W2.astype(f))[:, 0] + attn_b2.astype(f)[0]

    w_att = _sigmoid(s_table(tim_emb)[slot] - s_table(loc_emb)[lidx]).astype(f)

    # spatio-temporal weights (mirrors reference, f32)
    bsz = x.shape[0]
    bidx = np.arange(bsz)
    last = vl - 1
    maskf = (np.arange(l_steps)[None, :] < vl[:, None]).astype(f)
    last_t = t_sec[bidx, last]
    pad_t = t_sec * maskf
    minus = ((last_t[:, None] - pad_t) / f(86400.0)).astype(f)
    keep = minus != last_t[:, None]
    wgh = np.where(keep, np.exp(-minus * f(0.1)), f(0.0)).astype(f)
    last_s = spa[bidx, last]
    pad_s = spa * maskf[..., None]
    d = np.sqrt(np.sum((last_s[:, None, :] - pad_s) ** 2, axis=-1) + f(1e-12)).astype(f)
    keep_s = d != np.sqrt(np.sum(last_s * last_s, axis=-1) + f(1e-12))[:, None]
    wgh_s = np.where(keep_s, np.exp(-d * f(0.01)), f(0.0)).astype(f)
    stw = (wgh * wgh_s).astype(f)
    stw = stw / np.sum(stw, axis=1, keepdims=True)

    spa_base = spa_dis_mat_e[lidx[bidx, last]].astype(f) + fc_b.astype(f)[None, :]

    f16 = np.float16
    loc_emb16 = loc_emb.astype(f16)
    tim_emb16 = tim_emb.astype(f16)
    wihT16 = np.ascontiguousarray(gru_Wih.T).astype(f16)
    whhT16 = np.ascontiguousarray(gru_Whh.T).astype(f16)
    fcWT16 = np.ascontiguousarray(fc_W.T).astype(f16)

    in_maps = []
    for c in range(NC):
        bs = slice(c * BC, (c + 1) * BC)

        def wrap_idx(a):  # (t,b) order -> [16, items/16] int16
            flat = np.ascontiguousarray(a[bs].T).reshape(-1).astype(np.int16)
            return np.ascontiguousarray(flat.reshape(-1, 16).T)

        w_flat = np.ascontiguousarray(w_att[bs].T).reshape(1, -1).astype(f16)
        stw_flat = np.ascontiguousarray(stw[bs]).reshape(1, -1).astype(f16)
        spa_c = np.ascontiguousarray(
            spa_base[bs].reshape(BC, 4, NLOC // 4).transpose(1, 0, 2).reshape(128, NLOC // 4)
        )
        in_maps.append(
            dict(
                loc_emb16=loc_emb16,
                tim_emb16=tim_emb16,
                wihT16=wihT16,
                whhT16=whhT16,
                fcWT16=fcWT16,
                loc_idx=wrap_idx(lidx),
                tim_idx=wrap_idx(slot),
                w_att=np.ascontiguousarray(np.broadcast_to(w_flat, (128, l_steps * BC))),
                stw_b=np.ascontiguousarray(np.broadcast_to(stw_flat, (128, BC * l_steps))),
                spa4=spa_c,
            )
        )
    return in_maps


def run_device(in_maps, trace=False):
    from concourse.bass_utils import run_bass_kernel_spmd

    if "nc" not in _CACHE:
        _CACHE["nc"] = build_kernel()
    nc = _CACHE["nc"]
    res = run_bass_kernel_spmd(
        nc, in_maps, list(range(NC)), trace=trace,
        trace_cores=list(range(NC)) if trace else None,
    )
    return res


def kernel(**inputs):
    inputs = dict(inputs)
    inputs.pop("usrOneHot", None)
    in_maps = _prep(**inputs)
    res = run_device(in_maps, trace=False)
    out = np.empty((B, NLOC), np.float32)
    for c in range(NC):
        o4 = res.results[c]["out4"]
        out[c * BC : (c + 1) * BC] = (
            o4.reshape(4, BC, NLOC // 4).transpose(1, 0, 2).reshape(BC, NLOC)
        )
    return out


# revision 11
# speedup vs baseline: 1.2729x; 1.2729x over previous
"""Trainium2 Bass kernel for nn_LocationSemanticModel.

Data-parallel over batch: 8 cores x 32 batch each. Inside each core:
  - encoder: dma_gather(transpose) of loc/tim embeddings (f16, H-major),
    lerp with host-precomputed attention weights -> rep.T [128, 2, L*32]
  - GRU scan: GI = Wih@rep.T accumulated in PSUM per 8-step block (PE),
    recurrent Whh@h.T accumulates on top; gates on ACT/DVE/GPSIMD.
  - weighted hidden sum via big mul+reduce with host-precomputed stw.
  - fc: col-tiled matmul vs fc_W.T (f16) + host-gathered spa_dis rows.
No collectives; host concatenates per-core [32, NLOC] outputs.
"""

import os
import sys

import numpy as np

for _p in ("/opt/trn_rl_repo",):
    if _p not in sys.path and os.path.isdir(_p):
        sys.path.insert(0, _p)

import concourse.bass as bass  # noqa: E402
from concourse.bacc import Bacc  # noqa: E402
import concourse.mybir as mybir  # noqa: E402
from concourse.tile import TileContext  # noqa: E402

F16 = mybir.dt.float16
F32 = mybir.dt.float32
I16 = mybir.dt.int16

B, L, H, NLOC, NTIME = 256, 400, 256, 10000, 169
NC = 8
BC = B // NC          # 32 batch per core
HC = 2                # 256 = 2 x 128 hidden chunks
MC = 6                # 768 = 6 x 128 gate chunks
TB = 8                # scan block: GI steps per psum buffer
TE = 16               # encoder block steps: 16*32=512 idxs = gather limit
NFC = 500             # fc N-chunk (<=512 f32 psum bank)
AluOp = mybir.AluOpType
Act = mybir.ActivationFunctionType


def build_kernel(l_steps=L, bc=BC, debug=False):
    """Build the per-core SPMD Bass program. Returns nc."""
    nb = l_steps // TB            # scan blocks
    ne = l_steps // TE if l_steps % TE == 0 else 1   # encoder blocks
    te = TE if l_steps % TE == 0 else l_steps
    items = l_steps * bc
    nfc_chunks = NLOC // 4 // NFC  # 2500/500 = 5
    assert items % 128 == 0 and NLOC % 4 == 0 and (NLOC // 4) % NFC == 0

    nc = Bacc()

    # ---- DRAM parameters (per-core inputs) ----
    loc_emb = nc.declare_dram_parameter("loc_emb16", [NLOC, H], F16, isOutput=False)
    tim_emb = nc.declare_dram_parameter("tim_emb16", [NTIME, H], F16, isOutput=False)
    wihT = nc.declare_dram_parameter("wihT16", [H, 3 * H], F16, isOutput=False)
    whhT = nc.declare_dram_parameter("whhT16", [H, 3 * H], F16, isOutput=False)
    fcWT = nc.declare_dram_parameter("fcWT16", [H, NLOC], F16, isOutput=False)
    loc_idx = nc.declare_dram_parameter("loc_idx", [16, items // 16], I16, isOutput=False)
    tim_idx = nc.declare_dram_parameter("tim_idx", [16, items // 16], I16, isOutput=False)
    w_att = nc.declare_dram_parameter("w_att", [128, items], F16, isOutput=False)
    stw_b = nc.declare_dram_parameter("stw_b", [128, bc * l_steps], F16, isOutput=False)
    spa4 = nc.declare_dram_parameter("spa4", [128, NLOC // 4], F32, isOutput=False)
    out4 = nc.declare_dram_parameter("out4", [128, NLOC // 4], F32, isOutput=True)
    if debug:
        dbg_rep = nc.declare_dram_parameter("dbg_rep", [128, HC, l_steps * bc], F16, isOutput=True)
        dbg_hall = nc.declare_dram_parameter("dbg_hall", [128, HC, bc, l_steps], F16, isOutput=True)
        dbg_ss = nc.declare_dram_parameter("dbg_ss", [128, HC, bc], F32, isOutput=True)

    with TileContext(nc) as tc:
        with (
            tc.tile_pool(name="const", bufs=1) as cpool,
            tc.tile_pool(name="rep", bufs=1) as rep_pool,
            tc.tile_pool(name="hall", bufs=1) as hall_pool,
        ):
            # ---- persistent SBUF ----
            wih_sb = cpool.tile([128, HC, 3 * H], F16, tag="wih")
            whh_sb = cpool.tile([128, HC, 3 * H], F16, tag="whh")
            nc.sync.dma_start(
                out=wih_sb[:], in_=wihT.rearrange("(c p) m -> p c m", p=128)
            )
            nc.sync.dma_start(
                out=whh_sb[:], in_=whhT.rearrange("(c p) m -> p c m", p=128)
            )
            lidx_sb = cpool.tile([128, items // 16], I16, tag="lidx")
            tidx_sb = cpool.tile([128, items // 16], I16, tag="tidx")
            nc.vector.memset(lidx_sb[:], 0)
            nc.vector.memset(tidx_sb[:], 0)
            # sim reads idx wrap from partitions [0:16); HW (queue 0 tx
            # core) reads [16:32) — populate both.
            nc.sync.dma_start(out=lidx_sb[:16, :], in_=loc_idx[:, :])
            nc.sync.dma_start(out=tidx_sb[:16, :], in_=tim_idx[:, :])
            nc.sync.dma_start(out=lidx_sb[16:32, :], in_=loc_idx[:, :])
            nc.sync.dma_start(out=tidx_sb[16:32, :], in_=tim_idx[:, :])

            rep_sb = rep_pool.tile([128, HC, items], F16, tag="rep")
            h_all = hall_pool.tile([128, HC, bc, l_steps], F16, tag="hall")

            # ---- encoder: gather + lerp ----
            with (
                tc.tile_pool(name="enc", bufs=2) as epool,
                tc.tile_pool(name="encw", bufs=2) as wpool,
            ):
                for e in range(ne):
                    i0 = e * te * bc
                    n_i = te * bc
                    locT = epool.tile([128, HC, n_i], F16, tag="locT")
                    timT = epool.tile([128, HC, n_i], F16, tag="timT")
                    # dma_gather transpose crashes above 512 idxs/instr
                    assert n_i <= 512
                    nc.gpsimd.dma_gather(
                        out_ap=locT[:],
                        in_ap=loc_emb[:],
                        idxs_ap=lidx_sb[:, i0 // 16 : (i0 + n_i) // 16],
                        num_idxs=n_i,
                        num_idxs_reg=n_i,
                        elem_size=H,
                        transpose=True,
                    )
                    nc.gpsimd.dma_gather(
                        out_ap=timT[:],
                        in_ap=tim_emb[:],
                        idxs_ap=tidx_sb[:, i0 // 16 : (i0 + n_i) // 16],
                        num_idxs=n_i,
                        num_idxs_reg=n_i,
                        elem_size=H,
                        transpose=True,
                    )
                    w_sb = wpool.tile([128, n_i], F16, tag="w")
                    nc.sync.dma_start(out=w_sb[:], in_=w_att[:, i0 : i0 + n_i])
                    # rep = loc + w*(tim - loc)  (all DVE: keep gpsimd free
                    # for gather descriptor generation)
                    dlt = epool.tile([128, HC, n_i], F16, tag="dlt")
                    nc.vector.tensor_sub(out=dlt[:], in0=timT[:], in1=locT[:])
                    for hc in range(HC):
                        nc.vector.tensor_mul(
                            out=dlt[:, hc, :], in0=dlt[:, hc, :], in1=w_sb[:]
                        )
                    nc.vector.tensor_add(
                        out=rep_sb[:, :, i0 : i0 + n_i], in0=locT[:], in1=dlt[:]
                    )

            # ---- GRU scan ----
            # NG independent batch groups pipeline through the engines to
            # hide the per-step cross-engine dependency chain. The h state
            # lives directly in h_all (h2 written there; matmuls read the
            # t-1 column) — no copy.
            NG = 2
            gb = bc // NG
            with (
                tc.tile_pool(name="gi_ps", bufs=2, space="PSUM") as gi_pool,
                tc.tile_pool(name="hn_ps", bufs=2, space="PSUM") as hn_pool,
                tc.tile_pool(name="h0", bufs=1) as h0pool,
                tc.tile_pool(name="gate", bufs=4) as gpool,
            ):
                h0_sb = h0pool.tile([128, HC, bc], F16, tag="h0")
                nc.vector.memset(h0_sb[:], 0)

                def h_prev(g, t):
                    if t == 0:
                        return [h0_sb[:, kc, g * gb : (g + 1) * gb] for kc in range(HC)]
                    return [
                        h_all[:, kc, g * gb : (g + 1) * gb, t - 1] for kc in range(HC)
                    ]

                for kb in range(nb):
                    t0 = kb * TB
                    gi = gi_pool.tile([128, MC, TB * bc], F32, tag="gi", space="PSUM")
                    hn = hn_pool.tile([128, HC, TB * bc], F32, tag="hn", space="PSUM")
                    # GI block: gi[mc] += WihT[kc,mc].T @ rep[kc, block]
                    for mc in range(MC):
                        for kc in range(HC):
                            nc.tensor.matmul(
                                out=gi[:, mc, :],
                                lhsT=wih_sb[:, kc, mc * 128 : (mc + 1) * 128],
                                rhs=rep_sb[:, kc, t0 * bc : (t0 + TB) * bc],
                                start=(kc == 0),
                                stop=(kc == HC - 1),
                                skip_group_check=True,
                            )
                    for s in range(TB):
                        t = t0 + s
                        for g in range(NG):
                            o = s * bc + g * gb
                            sl = slice(o, o + gb)
                            hp = h_prev(g, t)
                            # recurrent matmuls: r,z accumulate into gi
                            for mc in range(MC):
                                dst = gi[:, mc, sl] if mc < 4 else hn[:, mc - 4, sl]
                                for kc in range(HC):
                                    nc.tensor.matmul(
                                        out=dst,
                                        lhsT=whh_sb[:, kc, mc * 128 : (mc + 1) * 128],
                                        rhs=hp[kc],
                                        start=(mc >= 4 and kc == 0),
                                        stop=(mc >= 4 and kc == HC - 1),
                                        skip_group_check=True,
                                    )
                            rz = gpool.tile([128, 4, gb], F16, tag=f"rz{g}")
                            nc.scalar.activation(rz[:], gi[:, 0:4, sl], Act.Sigmoid)
                            t1 = gpool.tile([128, HC, gb], F16, tag=f"t1{g}")
                            nc.vector.tensor_mul(
                                out=t1[:], in0=rz[:, 0:2, :], in1=hn[:, :, sl]
                            )
                            t2 = gpool.tile([128, HC, gb], F16, tag=f"t2{g}")
                            nc.vector.tensor_add(out=t2[:], in0=t1[:], in1=gi[:, 4:6, sl])
                            n16 = gpool.tile([128, HC, gb], F16, tag=f"n16{g}")
                            nc.scalar.activation(n16[:], t2[:], Act.Tanh)
                            # d' = z*h_prev (off critical path, runs during tanh)
                            d16 = gpool.tile([128, HC, gb], F16, tag=f"d16{g}")
                            for kc in range(HC):
                                nc.gpsimd.tensor_mul(
                                    out=d16[:, kc, :], in0=rz[:, 2 + kc, :], in1=hp[kc]
                                )
                            # w = (z-1)*n ; h2 = d' - w = z*h + (1-z)*n
                            w16 = gpool.tile([128, HC, gb], F16, tag=f"w16{g}")
                            nc.vector.scalar_tensor_tensor(
                                out=w16[:],
                                in0=rz[:, 2:4, :],
                                scalar=1.0,
                                in1=n16[:],
                                op0=AluOp.subtract,
                                op1=AluOp.mult,
                            )
                            nc.vector.tensor_sub(
                                out=h_all[:, :, g * gb : (g + 1) * gb, t],
                                in0=d16[:],
                                in1=w16[:],
                            )

            if debug:
                nc.sync.dma_start(out=dbg_rep[:], in_=rep_sb[:])
                nc.sync.dma_start(out=dbg_hall[:], in_=h_all[:])
            # ---- weighted sum + fc ----
            with (
                tc.tile_pool(name="fin", bufs=1) as fpool,
                tc.tile_pool(name="fc_ps", bufs=2, space="PSUM") as fc_pool,
            ):
                stw_sb = fpool.tile([128, bc, l_steps], F16, tag="stw")
                nc.sync.dma_start(out=stw_sb[:], in_=stw_b[:, :])
                fcw_sb = fpool.tile([128, HC, NLOC], F16, tag="fcw")
                nc.sync.dma_start(
                    out=fcw_sb[:], in_=fcWT.rearrange("(c p) m -> p c m", p=128)
                )
                spa_sb = fpool.tile([128, NLOC // 4], F32, tag="spa")
                nc.sync.dma_start(out=spa_sb[:], in_=spa4[:, :])

                for hc in range(HC):
                    nc.vector.tensor_mul(
                        out=h_all[:, hc, :, :], in0=h_all[:, hc, :, :], in1=stw_sb[:]
                    )
                ss = fpool.tile([128, HC, bc], F32, tag="ss")
                nc.vector.tensor_reduce(
                    out=ss[:], in_=h_all[:], axis=mybir.AxisListType.X, op=AluOp.add
                )
                if debug:
                    nc.sync.dma_start(out=dbg_ss[:], in_=ss[:])
                ss16 = fpool.tile([128, HC, bc], F16, tag="ss16")
                nc.vector.tensor_copy(out=ss16[:], in_=ss[:])

                out_sb = fpool.tile([128, NLOC // 4], F32, tag="out")
                for nck in range(nfc_chunks):
                    ps = fc_pool.tile([128, 512], F32, tag="fps", space="PSUM")
                    for kc in range(HC):
                        for j in range(4):
                            nc.tensor.matmul(
                                out=ps[32 * j : 32 * j + bc, :NFC],
                                lhsT=ss16[:, kc, :],
                                rhs=fcw_sb[
                                    :,
                                    kc,
                                    j * (NLOC // 4)
                                    + nck * NFC : j * (NLOC // 4)
                                    + (nck + 1) * NFC,
                                ],
                                start=(kc == 0),
                                stop=(kc == HC - 1),
                                tile_position=(0, 32 * j),
                                skip_group_check=True,
                            )
                    nsl = slice(nck * NFC, (nck + 1) * NFC)
                    nc.vector.tensor_add(out=out_sb[:, nsl], in0=spa_sb[:, nsl], in1=ps[:, :NFC])
                nc.sync.dma_start(out=out4[:, :], in_=out_sb[:])

    nc.finalize()
    return nc


# ------------------------- host side -------------------------

_CACHE = {}


def _sigmoid(x):
    return 1.0 / (1.0 + np.exp(-x))


def _prep(x, locOneHot, valLen, spa_dis_mat_e, loc_emb, tim_emb,
          attn_W1, attn_b1, attn_W2, attn_b2,
          gru_Wih, gru_Whh, gru_bih, gru_bhh, fc_W, fc_b):
    f = np.float32
    assert np.abs(gru_bih).max() == 0.0 and np.abs(gru_bhh).max() == 0.0, (
        "kernel assumes zero GRU biases"
    )
    l_steps = x.shape[1]
    slot = x[:, :, 4].astype(np.int32)
    lidx = locOneHot[:, :, 0].astype(np.int32)
    t_sec = x[:, :, 2].astype(f)
    spa = x[:, :, 0:2].astype(f)
    vl = valLen.astype(np.int32)

    # attention score tables (softmax over pair == sigmoid of diff)
    def s_table(emb):
        p = np.maximum(emb.astype(f) @ attn_W1.astype(f) + attn_b1.astype(f), 0.0)
        return (p @ attn_W2.astype(f))[:, 0] + attn_b2.astype(f)[0]

    w_att = _sigmoid(s_table(tim_emb)[slot] - s_table(loc_emb)[lidx]).astype(f)

    # spatio-temporal weights (mirrors reference, f32)
    bsz = x.shape[0]
    bidx = np.arange(bsz)
    last = vl - 1
    maskf = (np.arange(l_steps)[None, :] < vl[:, None]).astype(f)
    last_t = t_sec[bidx, last]
    pad_t = t_sec * maskf
    minus = ((last_t[:, None] - pad_t) / f(86400.0)).astype(f)
    keep = minus != last_t[:, None]
    wgh = np.where(keep, np.exp(-minus * f(0.1)), f(0.0)).astype(f)
    last_s = spa[bidx, last]
    pad_s = spa * maskf[..., None]
    d = np.sqrt(np.sum((last_s[:, None, :] - pad_s) ** 2, axis=-1) + f(1e-12)).astype(f)
    keep_s = d != np.sqrt(np.sum(last_s * last_s, axis=-1) + f(1e-12))[:, None]
    wgh_s = np.where(keep_s, np.exp(-d * f(0.01)), f(0.0)).astype(f)
    stw = (wgh * wgh_s).astype(f)
    stw = stw / np.sum(stw, axis=1, keepdims=True)

    spa_base = spa_dis_mat_e[lidx[bidx, last]].astype(f) + fc_b.astype(f)[None, :]

    f16 = np.float16
    loc_emb16 = loc_emb.astype(f16)
    tim_emb16 = tim_emb.astype(f16)
    wihT16 = np.ascontiguousarray(gru_Wih.T).astype(f16)
    whhT16 = np.ascontiguousarray(gru_Whh.T).astype(f16)
    fcWT16 = np.ascontiguousarray(fc_W.T).astype(f16)

    in_maps = []
    for c in range(NC):
        bs = slice(c * BC, (c + 1) * BC)

        def wrap_idx(a):  # (t,b) order -> [16, items/16] int16
            flat = np.ascontiguousarray(a[bs].T).reshape(-1).astype(np.int16)
            return np.ascontiguousarray(flat.reshape(-1, 16).T)

        w_flat = np.ascontiguousarray(w_att[bs].T).reshape(1, -1).astype(f16)
        stw_flat = np.ascontiguousarray(stw[bs]).reshape(1, -1).astype(f16)
        spa_c = np.ascontiguousarray(
            spa_base[bs].reshape(BC, 4, NLOC // 4).transpose(1, 0, 2).reshape(128, NLOC // 4)
        )
        in_maps.append(
            dict(
                loc_emb16=loc_emb16,
                tim_emb16=tim_emb16,
                wihT16=wihT16,
                whhT16=whhT16,
                fcWT16=fcWT16,
                loc_idx=wrap_idx(lidx),
                tim_idx=wrap_idx(slot),
                w_att=np.ascontiguousarray(np.broadcast_to(w_flat, (128, l_steps * BC))),
                stw_b=np.ascontiguousarray(np.broadcast_to(stw_flat, (128, BC * l_steps))),
                spa4=spa_c,
            )
        )
    return in_maps


def run_device(in_maps, trace=False):
    from concourse.bass_utils import run_bass_kernel_spmd

    if "nc" not in _CACHE:
        _CACHE["nc"] = build_kernel()
    nc = _CACHE["nc"]
    res = run_bass_kernel_spmd(
        nc, in_maps, list(range(NC)), trace=trace,
        trace_cores=list(range(NC)) if trace else None,
    )
    return res


def kernel(**inputs):
    inputs = dict(inputs)
    inputs.pop("usrOneHot", None)
    in_maps = _prep(**inputs)
    res = run_device(in_maps, trace=False)
    out = np.empty((B, NLOC), np.float32)
    for c in range(NC):
        o4 = res.results[c]["out4"]
        out[c * BC : (c + 1) * BC] = (
            o4.reshape(4, BC, NLOC // 4).transpose(1, 0, 2).reshape(BC, NLOC)
        )
    return out
